# revision 25
# baseline (speedup 1.0000x reference)
"""CoAttention Trainium2 kernel (v6: phase-split ramp + 1024-wide psum halves).

Computes A[b,i,j] = u[b,i,:]@w1 + v[b,j,:]@w2 + sum_d u[b,i,d]*w3[d]*v[b,j,d]
for u, v: [16, 2048, 256] f32, w1/w2/w3: [256] f32 -> A: [16, 2048, 2048] f32.

Key identity: A[i,j] = sum_d (u[i,d]*w3[d] + w2[d]) * vT[d,j] + w1u[i], i.e.
the w2v[j] term rides along the main contraction for free because w2[d] is
added to the stationary operand (as a per-partition ACT bias during the uw3t
PSUM->SBUF copy — zero extra PE cycles, no aug matmuls).

Per core (2 batches, data parallel across 8 cores; w1/w2/w3 replicated):
  - u[b], v[b] loaded via SWDGE cast-DMA (f32 HBM -> bf16 SBUF) in 8-block
    chunks, u before v, so both the u-prep and v-transpose chains start early
  - batch 0 defers the h1 (j 1024:2047) half-rows of i-blocks 0-7 until after
    all their h0 half-rows: the PE queue is in-order and h1 work would stall
    it on the second v load chunk while ready h0 work waits behind
  - vt[d, j] built via PE transpose in bf16; 8 transposes batched per
    [128,1024] PSUM bank -> single 1024-wide ACT copy out
  - bulk DVE prep per 4 i-blocks with 2D packed APs (w3/w1 pre-repeated 4x
    in SBUF so the ops stay in the DVE 2x perf mode): uw3 = u*w3 bf16,
    scr = u*w1 bf16, w1u = reduce(scr) bf16
  - per 4 i-blocks: 8 PE transposes of uw3 into one PSUM bank, 2 strided ACT
    bias-copies (bias=w2[chunk]) -> stationary lhsT = uw3T + w2
  - psum halves [128,1024] accumulated over 2 bf16 d-chunks = w3uv + w2v
  - finish: half A on DVE tensor_tensor(psum + w1u broadcast), half B on ACT
    activation(bias=w1u); both write fp16 (output precision traded for half
    the store bandwidth; rel-err gate is 2e-2)
  - 256KiB fp16 stores per half-row (sync HWDGE ring); host upcasts to f32
"""

import numpy as np
from contextlib import ExitStack

B, S, D = 16, 2048, 256
N_CORES = 8
BPC = B // N_CORES  # batches per core
P = 128
NB = S // P    # 16 seq blocks
NCH = D // P   # 2 contraction chunks
FH = 1024      # matmul free (moving) dim / finish width
NH = S // FH   # 2 j halves
GRP = 4        # i-blocks per prep group

_CACHE = {}


def _build():
    import concourse.bacc as bacc
    import concourse.mybir as mybir
    import concourse.tile as tile
    from concourse.masks import make_identity

    dt = mybir.dt
    f32 = dt.float32
    bf16 = dt.bfloat16
    f16 = dt.float16
    ADD = mybir.AluOpType.add
    MULT = mybir.AluOpType.mult
    IDENT = mybir.ActivationFunctionType.Identity

    nc = bacc.Bacc("TRN2", debug=False, num_devices=N_CORES)
    u_d = nc.dram_tensor("u", [BPC, S, D], f32, kind="ExternalInput").ap()
    v_d = nc.dram_tensor("v", [BPC, S, D], f32, kind="ExternalInput").ap()
    w1_d = nc.dram_tensor("w1", [1, D], f32, kind="ExternalInput").ap()
    w2_d = nc.dram_tensor("w2", [1, D], f32, kind="ExternalInput").ap()
    w3_d = nc.dram_tensor("w3", [1, D], f32, kind="ExternalInput").ap()
    out_d = nc.dram_tensor("out", [BPC, S, S], f16, kind="ExternalOutput").ap()

    with tile.TileContext(nc) as tc, ExitStack() as ctx:
        const = ctx.enter_context(tc.tile_pool(name="const", bufs=1))
        inp = ctx.enter_context(tc.tile_pool(name="inp", bufs=2))
        vt_pool = ctx.enter_context(tc.tile_pool(name="vt", bufs=2))
        work = ctx.enter_context(tc.tile_pool(name="work", bufs=3))
        outp = ctx.enter_context(tc.tile_pool(name="outp", bufs=4))
        pst = ctx.enter_context(tc.tile_pool(name="pst", bufs=1, space="PSUM"))
        psa = ctx.enter_context(tc.tile_pool(name="psa", bufs=3, space="PSUM"))

        # ---- constants ----
        identb = const.tile([P, P], bf16, tag="identb")
        make_identity(nc, identb[:])
        ones = const.tile([1, P], f32, tag="ones")
        nc.vector.memset(ones[:], 1.0)

        # HAM warmup: the PE clock is gated to 1.2 GHz until ~3.4us of
        # sustained activity. The PE would otherwise idle from preamble-end
        # until the first input chunk lands (~5us), so the first real
        # transposes+matmuls would run at half rate. These fillers depend
        # only on memset tiles, keeping the PE busy (and the clock warm)
        # through the load window. f32 runs at quarter rate: 512 cols = ~1.7us
        # each cold, so 3 of them span the window.
        junkrow = const.tile([1, FH], f32, tag="junkrow")
        nc.vector.memset(junkrow[:], 1.0)
        for i in range(3):
            ps = psa.tile([P, FH], f32, tag="ps", name=f"warmup_{i}")
            nc.tensor.matmul(
                ps[:, 0:512], lhsT=ones[:], rhs=junkrow[:, 0:512],
                start=True, stop=True,
            )

        w1r = const.tile([1, D], f32, tag="w1r")
        nc.scalar.dma_start(out=w1r[:], in_=w1_d)
        w2r = const.tile([1, D], f32, tag="w2r")
        nc.scalar.dma_start(out=w2r[:], in_=w2_d)
        w3r = const.tile([1, D], f32, tag="w3r")
        nc.scalar.dma_start(out=w3r[:], in_=w3_d)

        # w1/w3 broadcast across partitions, repeated GRP times along free
        # dim -> [128, GRP*256] bf16 (2D packed operands keep DVE in 2x mode)
        w1b4 = const.tile([P, GRP, D], bf16, tag="w1b4")
        w3b4 = const.tile([P, GRP, D], bf16, tag="w3b4")
        for wrow, wb in ((w1r, w1b4), (w3r, w3b4)):
            ps = psa.tile([P, FH], f32, tag="ps", name=f"psw_{wb.name}")
            nc.tensor.matmul(
                ps[:, :D], lhsT=ones[:], rhs=wrow[:], start=True, stop=True
            )
            for g in range(GRP):
                nc.vector.tensor_copy(wb[:, g, :], ps[:, :D])
        # w2 chunk columns [d_in_chunk, ch] f32: per-partition bias used to
        # fold w2 into the stationary operand during the uw3t copy.
        w2col = const.tile([P, NCH], f32, tag="w2col")
        for ch in range(NCH):
            ps = psa.tile([P, FH], f32, tag="ps", name=f"psw2_{ch}")
            nc.tensor.matmul(
                ps[:, 0:1], lhsT=w2r[:, ch * P:(ch + 1) * P],
                rhs=ones[:, 0:1], start=True, stop=True,
            )
            nc.vector.tensor_copy(w2col[:, ch:ch + 1], ps[:, 0:1])

        # input loads: SWDGE (gpsimd) cast f32->bf16 inside the DMA engines;
        # chunked+interleaved (u first) so downstream chains start early
        loads = []
        for bi in range(BPC):
            u_all = inp.tile([P, NB, D], bf16, tag="u_all")
            v_all = inp.tile([P, NB, D], bf16, tag="v_all")
            loads.append((v_all, u_all))
        HC = 8  # 8-block load chunks (4-block chunks measured slower: SWDGE
        # descriptor-gen serialization on the one Q7 ring outweighs the ramp)
        for bi in range(BPC):
            v_all, u_all = loads[bi]
            u_src = u_d[bi].rearrange("(nb p) d -> p nb d", p=P)
            v_src = v_d[bi].rearrange("(nb p) d -> p nb d", p=P)
            for h in range(NB // HC):
                hs = slice(h * HC, (h + 1) * HC)
                nc.gpsimd.dma_start(out=u_all[:, hs, :], in_=u_src[:, hs, :])
                nc.gpsimd.dma_start(out=v_all[:, hs, :], in_=v_src[:, hs, :])

        for bi in range(BPC):
            v_all, u_all = loads[bi]

            # transpose v -> vt [d_in_chunk, ch, j] bf16; 8 jb transposes of
            # the same chunk batched per PSUM bank, one 1024-wide copy out.
            # jh-outer: both chunks of the first 8 j-blocks transpose as soon
            # as the first v chunk lands (ch-outer would stall the in-order
            # PE queue on the second v chunk)
            vt = vt_pool.tile([P, NCH, S], bf16, tag="vt")

            def emit_vt(jh):
                for ch in range(NCH):
                    ps = pst.tile(
                        [P, FH], bf16, tag="pst", bufs=2,
                        name=f"pstv_{bi}_{ch}_{jh}",
                    )
                    for k in range(8):
                        jb = jh * 8 + k
                        nc.tensor.transpose(
                            ps[:, k * P:(k + 1) * P],
                            v_all[:, jb, ch * P:(ch + 1) * P],
                            identb[:],
                        )
                    nc.scalar.copy(
                        vt[:, ch, jh * FH:(jh + 1) * FH], ps[:]
                    )

            emit_vt(0)
            emit_vt(1)

            w1u_all = vt_pool.tile([P, NB], bf16, tag="w1u_all")
            uw3_all = vt_pool.tile([P, NB, D], bf16, tag="uw3_all")
            w1b4f = w1b4[:].rearrange("p a b -> p (a b)")
            w3b4f = w3b4[:].rearrange("p a b -> p (a b)")

            # per-group prep: bulk DVE ops (2D packed), then 8 PE transposes
            # into one PSUM bank, then 2 strided ACT bias-copies that fold w2
            # into the stationary operand.
            prep = {}

            def emit_prep(g):
                gs = slice(g * GRP, (g + 1) * GRP)
                u_flat = u_all[:, gs, :].rearrange("p a b -> p (a b)")
                nc.vector.tensor_tensor(
                    uw3_all[:, gs, :].rearrange("p a b -> p (a b)"),
                    u_flat, w3b4f, op=MULT,
                )
                scr4 = work.tile(
                    [P, GRP * D], bf16, tag="scr4", name=f"scr_{bi}_{g}"
                )
                nc.vector.tensor_tensor(scr4[:], u_flat, w1b4f, op=MULT)
                with nc.allow_low_precision(
                    reason="w1u in bf16: 0.06 abs err on a ~27-scale output"
                ):
                    nc.vector.tensor_reduce(
                        out=w1u_all[:, gs],
                        in_=scr4[:].rearrange("p (a b) -> p a b", a=GRP),
                        axis=mybir.AxisListType.X, op=ADD,
                    )
                uw3t4 = work.tile([P, GRP, NCH * P], bf16, tag="uw3t4")
                ps = pst.tile([P, FH], bf16, tag="pst", bufs=2, name=f"pst_u_{bi}_{g}")
                for k in range(GRP):
                    ib = g * GRP + k
                    for ch in range(NCH):
                        nc.tensor.transpose(
                            ps[:, k * D + ch * P:k * D + (ch + 1) * P],
                            uw3_all[:, ib, ch * P:(ch + 1) * P],
                            identb[:],
                        )
                ps3 = ps[:].rearrange("p (i c k) -> p i c k", i=GRP, c=NCH)
                for ch in range(NCH):
                    nc.scalar.activation(
                        out=uw3t4[:, :, ch * P:(ch + 1) * P],
                        in_=ps3[:, :, ch, :],
                        func=IDENT, bias=w2col[:, ch:ch + 1], scale=1.0,
                    )
                prep[g] = uw3t4

            # emission plan: batch 0 defers the h1 half-rows of i-blocks 0-7
            # until after their h0 half-rows — the PE queue is in-order, and
            # h1 matmuls would otherwise sit at its head stalled on the
            # second v load chunk while ready h0 work waits behind them.
            if bi == 0:
                plan = (
                    [(ib, 0) for ib in range(8)]
                    + [(ib, 1) for ib in range(8)]
                    + [(ib, h) for ib in range(8, NB) for h in range(NH)]
                )
            else:
                plan = [(ib, h) for ib in range(NB) for h in range(NH)]
            first_use = {}
            last_use = {}
            for idx, (ib, h) in enumerate(plan):
                g = ib // GRP
                first_use.setdefault(g, idx)
                last_use[g] = idx
            emit_at = {}
            for g in sorted(first_use):
                emit_at.setdefault(max(0, first_use[g] - 6), []).append(g)

            for idx, (ib, h) in enumerate(plan):
                g, k = divmod(ib, GRP)
                for eg in emit_at.get(idx, ()):
                    emit_prep(eg)
                uw3t4 = prep[g]
                w1u = w1u_all[:, ib:ib + 1]

                # one [128,1024] psum half per entry; evacuating engine
                # alternates by (ib+h) parity so DVE and ACT split evenly
                ps = psa.tile(
                    [P, FH], f32, tag="ps", name=f"ps_{bi}_{ib}_{h}"
                )
                # matmul PSUM writes are bank-limited to 512 f32: target the
                # two 512-wide halves of the psum tile separately
                for ch in range(NCH):
                    for q in range(2):
                        nc.tensor.matmul(
                            ps[:, q * 512:(q + 1) * 512],
                            lhsT=uw3t4[:, k, ch * P:(ch + 1) * P],
                            rhs=vt[:, ch, h * FH + q * 512:
                                   h * FH + (q + 1) * 512],
                            start=(ch == 0),
                            stop=(ch == NCH - 1),
                        )
                if (ib + h) % 2 == 0:
                    orow = outp.tile([P, FH], f16, tag="orow_a")
                    nc.vector.tensor_tensor(
                        orow[:], ps[:], w1u.broadcast_to([P, FH]), op=ADD,
                    )
                else:
                    orow = outp.tile([P, FH], f16, tag="orow_b")
                    nc.scalar.activation(
                        out=orow[:], in_=ps[:], func=IDENT,
                        bias=w1u, scale=1.0,
                    )
                rows = slice(ib * P, (ib + 1) * P)
                nc.sync.dma_start(
                    out=out_d[bi, rows, h * FH:(h + 1) * FH], in_=orow[:]
                )
                if idx == last_use[g] and g in prep:
                    del prep[g]

    nc.compile()
    return nc


def _get_nc():
    if "nc" not in _CACHE:
        _CACHE["nc"] = _build()
    return _CACHE["nc"]


def kernel(u, v, w1, w2, w3, _trace=False, _trace_cores=None, _results_out=None):
    from concourse.bass_utils import run_bass_kernel_spmd

    nc = _get_nc()
    u = np.ascontiguousarray(u, dtype=np.float32)
    v = np.ascontiguousarray(v, dtype=np.float32)
    w1 = np.ascontiguousarray(w1, dtype=np.float32).reshape(1, D)
    w2 = np.ascontiguousarray(w2, dtype=np.float32).reshape(1, D)
    w3 = np.ascontiguousarray(w3, dtype=np.float32).reshape(1, D)

    in_maps = [
        {
            "u": np.ascontiguousarray(u[c * BPC:(c + 1) * BPC]),
            "v": np.ascontiguousarray(v[c * BPC:(c + 1) * BPC]),
            "w1": w1,
            "w2": w2,
            "w3": w3,
        }
        for c in range(N_CORES)
    ]
    kw = {}
    if _trace:
        kw["trace"] = True
        if _trace_cores is not None:
            kw["trace_cores"] = _trace_cores
    res = run_bass_kernel_spmd(nc, in_maps, core_ids=list(range(N_CORES)), **kw)
    if _results_out is not None:
        _results_out.append(res)
    out = np.concatenate(
        [res.results[c]["out"] for c in range(N_CORES)], axis=0
    )
    return out.astype(np.float32)


# revision 26
# speedup vs baseline: 1.0416x; 1.0416x over previous
"""CoAttention Trainium2 kernel (v6: phase-split ramp + 1024-wide psum halves).

Computes A[b,i,j] = u[b,i,:]@w1 + v[b,j,:]@w2 + sum_d u[b,i,d]*w3[d]*v[b,j,d]
for u, v: [16, 2048, 256] f32, w1/w2/w3: [256] f32 -> A: [16, 2048, 2048] f32.

Key identity: A[i,j] = sum_d (u[i,d]*w3[d] + w2[d]) * vT[d,j] + w1u[i], i.e.
the w2v[j] term rides along the main contraction for free because w2[d] is
added to the stationary operand (as a per-partition ACT bias during the uw3t
PSUM->SBUF copy — zero extra PE cycles, no aug matmuls).

Per core (2 batches, data parallel across 8 cores; w1/w2/w3 replicated):
  - u[b], v[b] loaded via SWDGE cast-DMA (f32 HBM -> bf16 SBUF) in 8-block
    chunks, u before v, so both the u-prep and v-transpose chains start early
  - batch 0 defers the h1 (j 1024:2047) half-rows of i-blocks 0-7 until after
    all their h0 half-rows: the PE queue is in-order and h1 work would stall
    it on the second v load chunk while ready h0 work waits behind
  - vt[d, j] built via PE transpose in bf16; 8 transposes batched per
    [128,1024] PSUM bank -> single 1024-wide ACT copy out
  - bulk DVE prep per 4 i-blocks with 2D packed APs (w3/w1 pre-repeated 4x
    in SBUF so the ops stay in the DVE 2x perf mode): uw3 = u*w3 bf16,
    scr = u*w1 bf16, w1u = reduce(scr) bf16
  - per 4 i-blocks: 8 PE transposes of uw3 into one PSUM bank, 2 strided ACT
    bias-copies (bias=w2[chunk]) -> stationary lhsT = uw3T + w2
  - psum halves [128,1024] accumulated over 2 bf16 d-chunks = w3uv + w2v
  - finish: half A on DVE tensor_tensor(psum + w1u broadcast), half B on ACT
    activation(bias=w1u); both write fp16 (output precision traded for half
    the store bandwidth; rel-err gate is 2e-2)
  - 256KiB fp16 stores per half-row (sync HWDGE ring); host upcasts to f32
"""

import numpy as np
from contextlib import ExitStack

B, S, D = 16, 2048, 256
N_CORES = 8
BPC = B // N_CORES  # batches per core
P = 128
NB = S // P    # 16 seq blocks
NCH = D // P   # 2 contraction chunks
FH = 1024      # matmul free (moving) dim / finish width
NH = S // FH   # 2 j halves
GRP = 4        # i-blocks per prep group

_CACHE = {}


def _build():
    import concourse.bacc as bacc
    import concourse.mybir as mybir
    import concourse.tile as tile
    from concourse.masks import make_identity

    dt = mybir.dt
    f32 = dt.float32
    bf16 = dt.bfloat16
    f16 = dt.float16
    ADD = mybir.AluOpType.add
    MULT = mybir.AluOpType.mult
    IDENT = mybir.ActivationFunctionType.Identity

    nc = bacc.Bacc("TRN2", debug=False, num_devices=N_CORES)
    u_d = nc.dram_tensor("u", [BPC, S, D], f32, kind="ExternalInput").ap()
    v_d = nc.dram_tensor("v", [BPC, S, D], f32, kind="ExternalInput").ap()
    w1_d = nc.dram_tensor("w1", [1, D], f32, kind="ExternalInput").ap()
    w2_d = nc.dram_tensor("w2", [1, D], f32, kind="ExternalInput").ap()
    w3_d = nc.dram_tensor("w3", [1, D], f32, kind="ExternalInput").ap()
    out_d = nc.dram_tensor("out", [BPC, S, S], f16, kind="ExternalOutput").ap()

    with tile.TileContext(nc) as tc, ExitStack() as ctx:
        const = ctx.enter_context(tc.tile_pool(name="const", bufs=1))
        inp = ctx.enter_context(tc.tile_pool(name="inp", bufs=2))
        vt_pool = ctx.enter_context(tc.tile_pool(name="vt", bufs=2))
        work = ctx.enter_context(tc.tile_pool(name="work", bufs=3))
        outp = ctx.enter_context(tc.tile_pool(name="outp", bufs=4))
        pst = ctx.enter_context(tc.tile_pool(name="pst", bufs=1, space="PSUM"))
        psa = ctx.enter_context(tc.tile_pool(name="psa", bufs=3, space="PSUM"))

        # ---- constants ----
        identb = const.tile([P, P], bf16, tag="identb")
        make_identity(nc, identb[:])
        ones = const.tile([1, P], f32, tag="ones")
        nc.vector.memset(ones[:], 1.0)

        # (HAM-warmup filler matmuls here were measured a net loss: they sit
        # ahead of the w1/w3 broadcast matmuls in the in-order PE queue and
        # delay the first prep; the cold-clock ops they would warm all run
        # inside load-bound dead time anyway.)
        w1r = const.tile([1, D], f32, tag="w1r")
        nc.scalar.dma_start(out=w1r[:], in_=w1_d)
        w2r = const.tile([1, D], f32, tag="w2r")
        nc.scalar.dma_start(out=w2r[:], in_=w2_d)
        w3r = const.tile([1, D], f32, tag="w3r")
        nc.scalar.dma_start(out=w3r[:], in_=w3_d)

        # w1/w3 broadcast across partitions, repeated GRP times along free
        # dim -> [128, GRP*256] bf16 (2D packed operands keep DVE in 2x mode)
        w1b4 = const.tile([P, GRP, D], bf16, tag="w1b4")
        w3b4 = const.tile([P, GRP, D], bf16, tag="w3b4")
        for wrow, wb in ((w1r, w1b4), (w3r, w3b4)):
            ps = psa.tile([P, FH], f32, tag="ps", name=f"psw_{wb.name}")
            nc.tensor.matmul(
                ps[:, :D], lhsT=ones[:], rhs=wrow[:], start=True, stop=True
            )
            for g in range(GRP):
                nc.vector.tensor_copy(wb[:, g, :], ps[:, :D])
        # w2 chunk columns [d_in_chunk, ch] f32: per-partition bias used to
        # fold w2 into the stationary operand during the uw3t copy.
        w2col = const.tile([P, NCH], f32, tag="w2col")
        for ch in range(NCH):
            ps = psa.tile([P, FH], f32, tag="ps", name=f"psw2_{ch}")
            nc.tensor.matmul(
                ps[:, 0:1], lhsT=w2r[:, ch * P:(ch + 1) * P],
                rhs=ones[:, 0:1], start=True, stop=True,
            )
            nc.vector.tensor_copy(w2col[:, ch:ch + 1], ps[:, 0:1])

        # input loads: SWDGE (gpsimd) cast f32->bf16 inside the DMA engines;
        # chunked+interleaved (u first) so downstream chains start early
        loads = []
        for bi in range(BPC):
            u_all = inp.tile([P, NB, D], bf16, tag="u_all")
            v_all = inp.tile([P, NB, D], bf16, tag="v_all")
            loads.append((v_all, u_all))
        HC = 8  # 8-block load chunks (4-block chunks measured slower: SWDGE
        # descriptor-gen serialization on the one Q7 ring outweighs the ramp)
        for bi in range(BPC):
            v_all, u_all = loads[bi]
            u_src = u_d[bi].rearrange("(nb p) d -> p nb d", p=P)
            v_src = v_d[bi].rearrange("(nb p) d -> p nb d", p=P)
            for h in range(NB // HC):
                hs = slice(h * HC, (h + 1) * HC)
                nc.gpsimd.dma_start(out=u_all[:, hs, :], in_=u_src[:, hs, :])
                nc.gpsimd.dma_start(out=v_all[:, hs, :], in_=v_src[:, hs, :])

        for bi in range(BPC):
            v_all, u_all = loads[bi]

            # transpose v -> vt [d_in_chunk, ch, j] bf16; 8 jb transposes of
            # the same chunk batched per PSUM bank, one 1024-wide copy out.
            # jh-outer: both chunks of the first 8 j-blocks transpose as soon
            # as the first v chunk lands (ch-outer would stall the in-order
            # PE queue on the second v chunk)
            vt = vt_pool.tile([P, NCH, S], bf16, tag="vt")

            def emit_vt(jh):
                for ch in range(NCH):
                    ps = pst.tile(
                        [P, FH], bf16, tag="pst", bufs=2,
                        name=f"pstv_{bi}_{ch}_{jh}",
                    )
                    for k in range(8):
                        jb = jh * 8 + k
                        nc.tensor.transpose(
                            ps[:, k * P:(k + 1) * P],
                            v_all[:, jb, ch * P:(ch + 1) * P],
                            identb[:],
                        )
                    nc.scalar.copy(
                        vt[:, ch, jh * FH:(jh + 1) * FH], ps[:]
                    )

            emit_vt(0)
            emit_vt(1)

            w1u_all = vt_pool.tile([P, NB], bf16, tag="w1u_all")
            uw3_all = vt_pool.tile([P, NB, D], bf16, tag="uw3_all")
            w1b4f = w1b4[:].rearrange("p a b -> p (a b)")
            w3b4f = w3b4[:].rearrange("p a b -> p (a b)")

            # per-group prep: bulk DVE ops (2D packed), then 8 PE transposes
            # into one PSUM bank, then 2 strided ACT bias-copies that fold w2
            # into the stationary operand.
            prep = {}

            def emit_prep(g):
                gs = slice(g * GRP, (g + 1) * GRP)
                u_flat = u_all[:, gs, :].rearrange("p a b -> p (a b)")
                nc.vector.tensor_tensor(
                    uw3_all[:, gs, :].rearrange("p a b -> p (a b)"),
                    u_flat, w3b4f, op=MULT,
                )
                scr4 = work.tile(
                    [P, GRP * D], bf16, tag="scr4", name=f"scr_{bi}_{g}"
                )
                nc.vector.tensor_tensor(scr4[:], u_flat, w1b4f, op=MULT)
                with nc.allow_low_precision(
                    reason="w1u in bf16: 0.06 abs err on a ~27-scale output"
                ):
                    nc.vector.tensor_reduce(
                        out=w1u_all[:, gs],
                        in_=scr4[:].rearrange("p (a b) -> p a b", a=GRP),
                        axis=mybir.AxisListType.X, op=ADD,
                    )
                uw3t4 = work.tile([P, GRP, NCH * P], bf16, tag="uw3t4")
                ps = pst.tile([P, FH], bf16, tag="pst", bufs=2, name=f"pst_u_{bi}_{g}")
                for k in range(GRP):
                    ib = g * GRP + k
                    for ch in range(NCH):
                        nc.tensor.transpose(
                            ps[:, k * D + ch * P:k * D + (ch + 1) * P],
                            uw3_all[:, ib, ch * P:(ch + 1) * P],
                            identb[:],
                        )
                ps3 = ps[:].rearrange("p (i c k) -> p i c k", i=GRP, c=NCH)
                for ch in range(NCH):
                    nc.scalar.activation(
                        out=uw3t4[:, :, ch * P:(ch + 1) * P],
                        in_=ps3[:, :, ch, :],
                        func=IDENT, bias=w2col[:, ch:ch + 1], scale=1.0,
                    )
                prep[g] = uw3t4

            # emission plan: batch 0 defers the h1 half-rows of i-blocks 0-7
            # until after their h0 half-rows — the PE queue is in-order, and
            # h1 matmuls would otherwise sit at its head stalled on the
            # second v load chunk while ready h0 work waits behind them.
            if bi == 0:
                plan = (
                    [(ib, 0) for ib in range(8)]
                    + [(ib, 1) for ib in range(8)]
                    + [(ib, h) for ib in range(8, NB) for h in range(NH)]
                )
            else:
                plan = [(ib, h) for ib in range(NB) for h in range(NH)]
            first_use = {}
            last_use = {}
            for idx, (ib, h) in enumerate(plan):
                g = ib // GRP
                first_use.setdefault(g, idx)
                last_use[g] = idx
            emit_at = {}
            for g in sorted(first_use):
                emit_at.setdefault(max(0, first_use[g] - 6), []).append(g)

            for idx, (ib, h) in enumerate(plan):
                g, k = divmod(ib, GRP)
                for eg in emit_at.get(idx, ()):
                    emit_prep(eg)
                uw3t4 = prep[g]
                w1u = w1u_all[:, ib:ib + 1]

                # one [128,1024] psum half per entry; evacuating engine
                # alternates by (ib+h) parity so DVE and ACT split evenly
                ps = psa.tile(
                    [P, FH], f32, tag="ps", name=f"ps_{bi}_{ib}_{h}"
                )
                # matmul PSUM writes are bank-limited to 512 f32: target the
                # two 512-wide halves of the psum tile separately
                for ch in range(NCH):
                    for q in range(2):
                        nc.tensor.matmul(
                            ps[:, q * 512:(q + 1) * 512],
                            lhsT=uw3t4[:, k, ch * P:(ch + 1) * P],
                            rhs=vt[:, ch, h * FH + q * 512:
                                   h * FH + (q + 1) * 512],
                            start=(ch == 0),
                            stop=(ch == NCH - 1),
                        )
                if (ib + h) % 2 == 0:
                    orow = outp.tile([P, FH], f16, tag="orow_a")
                    nc.vector.tensor_tensor(
                        orow[:], ps[:], w1u.broadcast_to([P, FH]), op=ADD,
                    )
                else:
                    orow = outp.tile([P, FH], f16, tag="orow_b")
                    nc.scalar.activation(
                        out=orow[:], in_=ps[:], func=IDENT,
                        bias=w1u, scale=1.0,
                    )
                rows = slice(ib * P, (ib + 1) * P)
                nc.sync.dma_start(
                    out=out_d[bi, rows, h * FH:(h + 1) * FH], in_=orow[:]
                )
                if idx == last_use[g] and g in prep:
                    del prep[g]

    nc.compile()
    return nc


def _get_nc():
    if "nc" not in _CACHE:
        _CACHE["nc"] = _build()
    return _CACHE["nc"]


def kernel(u, v, w1, w2, w3, _trace=False, _trace_cores=None, _results_out=None):
    from concourse.bass_utils import run_bass_kernel_spmd

    nc = _get_nc()
    u = np.ascontiguousarray(u, dtype=np.float32)
    v = np.ascontiguousarray(v, dtype=np.float32)
    w1 = np.ascontiguousarray(w1, dtype=np.float32).reshape(1, D)
    w2 = np.ascontiguousarray(w2, dtype=np.float32).reshape(1, D)
    w3 = np.ascontiguousarray(w3, dtype=np.float32).reshape(1, D)

    in_maps = [
        {
            "u": np.ascontiguousarray(u[c * BPC:(c + 1) * BPC]),
            "v": np.ascontiguousarray(v[c * BPC:(c + 1) * BPC]),
            "w1": w1,
            "w2": w2,
            "w3": w3,
        }
        for c in range(N_CORES)
    ]
    kw = {}
    if _trace:
        kw["trace"] = True
        if _trace_cores is not None:
            kw["trace_cores"] = _trace_cores
    res = run_bass_kernel_spmd(nc, in_maps, core_ids=list(range(N_CORES)), **kw)
    if _results_out is not None:
        _results_out.append(res)
    out = np.concatenate(
        [res.results[c]["out"] for c in range(N_CORES)], axis=0
    )
    return out.astype(np.float32)
